# revision 15
# baseline (speedup 1.0000x reference)
"""Bass/Trainium2 kernel for nn_DWAMiddleLayer (low-rank MoE weight-assembly layer).

Math (reference):
    U    = pool[:, :1024].reshape(N, DB, R)      # [512, 256, 4]
    V    = pool[:, 1024:2048].reshape(N, R, DA)  # [512, 4, 256]
    bE   = pool[:, 2048:2304]                    # [512, 256]
    h_t  = h_A @ W_base.T
           + sum_r (alpha * (h_A @ V_r.T)) @ U_r          # never materialize W_assembled
           + alpha @ bE + b_base
    y    = h_A + gamma * h_t ; out = LayerNorm(y) * ln_scale + ln_bias

Distribution: data-parallel over batch B=2048 across 8 cores (BS=256 rows each);
pool/W_base/vectors replicated.

v5: the whole h_t matmul path runs in fp8-e4m3 DoubleRow (2 k-tiles per
instruction, 2x PE rate, half the pool HBM bytes) into ONE accumulator that
carries a 128x power-of-2 scale (V*64, U*32, alpha*2^-4; bE*128, W_base*128);
the epilogue divides it out during the PSUM->SBUF copy. gamma*b_base is folded
into the fp32 residual h_A on the host (exact), removing the rank-1 matmul.
gamma-scaling of h_t keeps the fp8 error ~1e-3 in the output. Bulk data flows
via SWDGE (gpsimd, ~340GB/s); sa via the sync HWDGE ring in parallel. The PE
is warmed with dummy matmuls until real data arrives so the HAM clock gate
(1.2 vs 2.4 GHz) stays lifted. LN epilogue is pipelined per batch-chunk across
Scalar (normalize via per-partition scale/bias) and Vector, with per-chunk
output DMAs on both HWDGE rings.
"""

import numpy as np

B, N, D_A, D_B, R = 2048, 512, 256, 256, 4
NC_COUNT = 8
BS = B // NC_COUNT  # 256 batch rows per core
P = 128
LN_EPS = 1e-5

V_SCALE = 64.0
U_SCALE = 32.0
A_SCALE = 1.0 / 16.0       # alpha^T pre-scale for the s-path
W_SCALE = 128.0            # W_base^T and bE fp8 scales (match the accumulator)
ACC_SCALE = V_SCALE * U_SCALE * A_SCALE  # = 128: acc8 carries 128 * h_t

# ---- sa1 (bf16 cols; fp8 regions bitcast), needed first (HWDGE) ----
SA_HA8 = 0      # hA^T fp8         [p_a, 2 ach, 256 b]  (256 carrier cols)
SA_ALT = 256    # alpha^T * 2^-4 bf16 [p_n, 4 och, 256 b]
SA_GE = 1280    # fp32 [gamma, eps] bitcast -> 4 bf16 cols
SA_W = 1284
# ---- sa2 (bf16 cols; fp8 regions bitcast), needed mid-stream (SWDGE) ----
S2_A8 = 0       # alpha^T fp8      [p_n, 4 och, 256 b]  (512 carrier cols)
S2_ID = 512     # ident            [p, 128] bf16
S2_WB8 = 640    # W_base^T * 128 fp8 [p_a, 2 ach, 256 c] (256 carrier cols)
S2_BE8 = 896    # bE * 128 fp8     [p_n, 4 o, 256 c]    (512 carrier cols)
S2_W = 1408
# ---- packed small tensor B (bf16 cols), needed late (epilogue) ----
SB_HAF = 0      # (h_A + gamma*b_base) fp32 [p_b, 2 bch, 256 a] -> 1024 bf16 cols
SB_LSC = 1024   # ln_scale  [p, 256] replicated
SB_LBI = 1280   # ln_bias   [p, 256] replicated
SB_W = 1536
# ---- fp8 pool pair layout: d_p8 bf16 [128, 2 pair, 2048] -> fp8 [., ., 4096]
#      per o within pair: [VT 1024 | U2 1024] fp8 cols
P8_VT = 0       # [ach(2), r(4), pn(128)]
P8_U2 = 1024    # [r(4), cch(2), pc(128)]

N_WARM = 12  # warm-up matmuls (j=512): bridge PE activity until data arrives

_cache = {}


def _build_nc():
    import concourse.mybir as mybir
    import concourse.tile as tile
    from concourse import bacc

    fp32 = mybir.dt.float32
    bf16 = mybir.dt.bfloat16
    fp8 = mybir.dt.float8e4
    DR = mybir.MatmulPerfMode.DoubleRow

    nc = bacc.Bacc("TRN2", target_bir_lowering=False)

    # ---- DRAM I/O (per-core shard shapes) ----
    d_sa = nc.dram_tensor("sma", [P, SA_W], bf16, kind="ExternalInput")
    d_sa2 = nc.dram_tensor("sma2", [P, S2_W], bf16, kind="ExternalInput")
    d_p8 = nc.dram_tensor("p8", [P, 2, 2048], bf16, kind="ExternalInput")
    d_sb = nc.dram_tensor("smb", [P, SB_W], bf16, kind="ExternalInput")
    d_out = nc.dram_tensor("out", [BS, D_A], fp32, kind="ExternalOutput")

    with tile.TileContext(nc) as tc:
        with (
            tc.tile_pool(name="persist", bufs=1) as persist,
            tc.tile_pool(name="stage", bufs=2) as stage,
            tc.tile_pool(name="sm", bufs=3) as sm,
            tc.tile_pool(name="pp_t", bufs=2, space="PSUM") as pp_t,
            tc.tile_pool(name="pp_a8", bufs=1, space="PSUM") as pp_a8,
            tc.tile_pool(name="pp_tr", bufs=1, space="PSUM") as pp_tr,
            tc.tile_pool(name="pp_w", bufs=1, space="PSUM") as pp_w,
        ):
            # ---------- PE warm-up: junk matmuls to lift the HAM clock gate ----------
            wsrc = persist.tile([P, 512], bf16)
            nc.vector.memset(wsrc, 0.0)
            warm_ps = pp_w.tile([P, 512], fp32, tag="warm")
            for _ in range(N_WARM):
                nc.tensor.matmul(
                    warm_ps, lhsT=wsrc[:, 0:P], rhs=wsrc, start=True, stop=True,
                    skip_group_check=True,
                )

            # ---------- loads ----------
            # bulk fp8 pool pairs + late smalls via SWDGE (gpsimd, FIFO order);
            # sa via the sync HWDGE ring concurrently.
            p8t = [
                stage.tile([P, 2048], bf16, tag="p8", name=f"p8_{pr}")
                for pr in range(2)
            ]
            # SWDGE (FIFO): pair1, then sa2, then sb (none needed first)
            nc.gpsimd.dma_start(p8t[1], d_p8[:, 1])
            sa2 = persist.tile([P, S2_W], bf16)
            nc.gpsimd.dma_start(sa2, d_sa2[:])
            sb = persist.tile([P, SB_W], bf16)
            nc.gpsimd.dma_start(sb, d_sb[:])
            # HWDGE (fast first-byte): sa1 then pair0 -- the critical-path data
            sa = persist.tile([P, SA_W], bf16)
            nc.sync.dma_start(sa, d_sa[:])
            nc.sync.dma_start(p8t[0], d_p8[:, 0])

            hA8 = sa[:, SA_HA8 : SA_HA8 + 256].bitcast(fp8).rearrange(
                "p (a b) -> p a b", a=2
            )
            alphaT = sa[:, SA_ALT : SA_ALT + 1024].rearrange("p (o b) -> p o b", o=4)
            a8 = sa2[:, S2_A8 : S2_A8 + 512].bitcast(fp8).rearrange(
                "p (o b) -> p o b", o=4
            )
            ident_b = sa2[:, S2_ID : S2_ID + P]
            Wb8 = sa2[:, S2_WB8 : S2_WB8 + 256].bitcast(fp8).rearrange(
                "p (a c) -> p a c", a=2
            )
            bE8 = sa2[:, S2_BE8 : S2_BE8 + 512].bitcast(fp8).rearrange(
                "p (o c) -> p o c", o=4
            )
            ge = sa[:, SA_GE : SA_GE + 4].bitcast(fp32)
            gamma_col = ge[:, 0:1]
            eps_col = ge[:, 1:2]
            hA_f32 = sb[:, SB_HAF : SB_HAF + 1024].bitcast(fp32).rearrange(
                "p (o a) -> p o a", o=2
            )
            lsc_row = sb[:, SB_LSC : SB_LSC + 256]
            lbi_row = sb[:, SB_LBI : SB_LBI + 256]

            # warm the ACT tables (Copy for the copies, Sqrt for the LN tail)
            warm_act = sm.tile([P, 1], fp32, tag="warmact")
            nc.scalar.activation(
                warm_act, wsrc[:, 0:1], mybir.ActivationFunctionType.Copy
            )
            nc.scalar.activation(
                warm_act, wsrc[:, 0:1], mybir.ActivationFunctionType.Sqrt
            )

            # ---------- h_t^T accumulator (fp8 DoubleRow path, x128 scale) ----------
            acc8 = pp_a8.tile([P, 2, BS], fp32, tag="a8")
            st8 = [False, False]

            def mm8(ch, lhsT, rhs, last=False):
                nc.tensor.matmul(
                    acc8[:, ch], lhsT=lhsT, rhs=rhs,
                    start=(not st8[ch]), stop=last,
                    perf_mode=DR, skip_group_check=True,
                )
                st8[ch] = True

            # ---------- main pipeline over expert-chunk pairs ----------
            for pr in range(2):
                pc8 = p8t[pr].bitcast(fp8).rearrange("p (o f) -> p o f", o=2)
                s8 = sm.tile([P, 2, 4, BS], fp8, tag="s8")
                for oi in range(2):
                    o = pr * 2 + oi
                    VT_o = pc8[:, oi, P8_VT : P8_VT + 1024].rearrange(
                        "p (a r q) -> p a r q", a=2, r=4
                    )
                    # mm1 (DoubleRow, contraction a=256 in one matmul per r)
                    t_ps = pp_t.tile([P, 4, BS], fp32, tag="t")
                    for r in range(4):
                        nc.tensor.matmul(
                            t_ps[:, r],
                            lhsT=VT_o[:, :, r],
                            rhs=hA8,
                            start=True,
                            stop=True,
                            perf_mode=DR,
                        )
                    # s = (alpha * 2^-4) * t : direct-from-PSUM DVE multiply
                    nc.vector.tensor_mul(
                        s8[:, oi], t_ps,
                        alphaT[:, o : o + 1, :].to_broadcast((P, 4, BS)),
                    )
                # bias-mm (DoubleRow): 128*bE^T @ alpha^T, contraction n-pair
                for ch in range(2):
                    mm8(ch, bE8[:, 2 * pr : 2 * pr + 2, ch * P : (ch + 1) * P],
                        a8[:, 2 * pr : 2 * pr + 2])
                if pr == 0:
                    # base-mm (DoubleRow): 128*W_base^T @ hA^T, contraction a
                    for ch in range(2):
                        mm8(ch, Wb8[:, :, ch * P : (ch + 1) * P], hA8)
                # mm2 (DoubleRow over the o-pair, contraction n=256)
                U2_pr = pc8[:, :, P8_U2 : P8_U2 + 1024].rearrange(
                    "p o (r c q) -> p o r c q", r=4, c=2
                )
                for r in range(4):
                    for ch in range(2):
                        mm8(ch, U2_pr[:, :, r, ch], s8[:, :, r],
                            last=(pr == 1 and r == 3 and ch == 1))

            # ---------- epilogue: h_t back to batch-major, residual + LN ----------
            ht8 = sm.tile([P, 2, BS], bf16, tag="ht8")
            nc.scalar.activation(
                ht8, acc8, mybir.ActivationFunctionType.Copy, scale=1.0 / ACC_SCALE
            )
            ht_ps = pp_tr.tile([P, 2, D_A], fp32, tag="tr")
            for bch in range(2):
                for cch in range(2):
                    nc.tensor.matmul(
                        ht_ps[:, bch, cch * P : (cch + 1) * P],
                        lhsT=ht8[:, cch, bch * P : (bch + 1) * P],
                        rhs=ident_b,
                        start=True,
                        stop=True,
                        skip_group_check=True,
                    )

            # y = (h_A + gamma*b_base) + gamma * h_t', per-batch-chunk pipeline
            y_sb = sm.tile([P, 2, D_A], fp32, tag="y")
            stats = sm.tile([P, 2, 6], fp32, tag="st")
            mv = sm.tile([P, 2, 2], fp32, tag="mv")
            for bch in range(2):
                nc.vector.scalar_tensor_tensor(
                    y_sb[:, bch],
                    in0=ht_ps[:, bch],
                    scalar=gamma_col,
                    in1=hA_f32[:, bch],
                    op0=mybir.AluOpType.mult,
                    op1=mybir.AluOpType.add,
                )
                nc.vector.bn_stats(stats[:, bch], y_sb[:, bch])
                nc.vector.bn_aggr(mv[:, bch], stats[:, bch])
            # per-batch-chunk: rstd/nmr, normalize on ACT, scale/bias on DVE
            rstd = sm.tile([P, 2], fp32, tag="rstd")
            nmr = sm.tile([P, 2], fp32, tag="nmr")
            w_sb = sm.tile([P, 2, D_A], fp32, tag="w")
            out_sb = sm.tile([P, 2, D_A], fp32, tag="out")
            for bch in range(2):
                nc.scalar.activation(
                    rstd[:, bch : bch + 1],
                    mv[:, bch, 1:2],
                    mybir.ActivationFunctionType.Sqrt,
                    bias=eps_col,
                )
                nc.vector.reciprocal(rstd[:, bch : bch + 1], rstd[:, bch : bch + 1])
                nc.vector.scalar_tensor_tensor(
                    nmr[:, bch : bch + 1],
                    in0=mv[:, bch, 0:1],
                    scalar=-1.0,
                    in1=rstd[:, bch : bch + 1],
                    op0=mybir.AluOpType.mult,
                    op1=mybir.AluOpType.mult,
                )
                nc.scalar.activation(
                    w_sb[:, bch],
                    y_sb[:, bch],
                    mybir.ActivationFunctionType.Identity,
                    bias=nmr[:, bch : bch + 1],
                    scale=rstd[:, bch : bch + 1],
                )
                nc.vector.tensor_mul(w_sb[:, bch], w_sb[:, bch], lsc_row)
                nc.vector.tensor_add(out_sb[:, bch], w_sb[:, bch], lbi_row)
                eng = nc.sync if bch == 0 else nc.scalar
                eng.dma_start(d_out[bch * P : (bch + 1) * P, :], out_sb[:, bch])

    nc.compile()
    return nc


def _get_nc():
    if "nc" not in _cache:
        _cache["nc"] = _build_nc()
    return _cache["nc"]


def make_in_maps(**inputs):
    """Shard + pre-transpose + pre-cast full inputs into 8 per-core input maps."""
    import ml_dtypes

    bf = ml_dtypes.bfloat16
    f8 = ml_dtypes.float8_e4m3fn
    f32 = lambda x: np.ascontiguousarray(np.asarray(x), dtype=np.float32)

    def to8c(x):  # fp8 bytes packed into a bf16 bit-carrier, 2 per column
        q = np.clip(x, -240.0, 240.0).astype(f8)  # TRN e4m3 tops out at +-240
        return q.reshape(q.shape[0], -1).view(np.uint8).view(np.uint16).view(bf)

    h_A = f32(inputs["h_A"])
    alpha = f32(inputs["alpha"])
    pool = np.asarray(inputs["pool_vectors"], dtype=np.float32)
    W_base = f32(inputs["W_base"])
    b_base = f32(inputs["b_base"]).reshape(D_B)
    gamma = float(np.asarray(inputs["gamma"]).reshape(()))
    ln_scale = f32(inputs["ln_scale"]).reshape(D_A)
    ln_bias = f32(inputs["ln_bias"]).reshape(D_A)

    U = pool[:, : D_B * R].reshape(N, D_B, R)
    V = pool[:, D_B * R : D_B * R + R * D_A].reshape(N, R, D_A)
    bE = pool[:, D_B * R + R * D_A : D_B * R + R * D_A + D_B]

    # fp8 pool pairs: [p, pair, o_in_pair, [VT | U2]] packed as bf16 bit-carrier
    p8 = np.empty((P, 2, 2, 2048), np.float32)
    for o in range(4):
        nsl = slice(o * P, (o + 1) * P)
        vt = V[nsl].transpose(2, 1, 0).reshape(2, P, R, P).transpose(1, 0, 2, 3)
        p8[:, o // 2, o % 2, P8_VT : P8_VT + 1024] = vt.reshape(P, 1024) * V_SCALE
        u2 = U[nsl].transpose(0, 2, 1).reshape(P, R, 2, P)
        p8[:, o // 2, o % 2, P8_U2 : P8_U2 + 1024] = u2.reshape(P, 1024) * U_SCALE
    p8_carrier = to8c(p8.reshape(P, -1)).reshape(P, 2, 2048)

    ident = np.eye(P, dtype=np.float32).astype(bf)
    ge = np.empty((P, 2), np.float32)
    ge[:, 0] = gamma
    ge[:, 1] = LN_EPS
    wbt = np.ascontiguousarray(
        W_base.T.reshape(2, P, D_B).transpose(1, 0, 2).reshape(P, 512)
    )
    be = np.ascontiguousarray(
        bE.reshape(4, P, D_B).transpose(1, 0, 2).reshape(P, 1024)
    )

    in_maps = []
    for i in range(NC_COUNT):
        sl = slice(i * BS, (i + 1) * BS)
        hat = h_A[sl].T.reshape(2, P, BS).transpose(1, 0, 2).reshape(P, 512)
        alt = alpha[sl].T.reshape(4, P, BS).transpose(1, 0, 2).reshape(P, 1024)

        sa = np.zeros((P, SA_W), bf)
        sa[:, SA_HA8 : SA_HA8 + 256] = to8c(hat)
        sa[:, SA_ALT : SA_ALT + 1024] = (alt * A_SCALE).astype(bf)
        sa[:, SA_GE : SA_GE + 4] = ge.view(bf)
        sa2 = np.zeros((P, S2_W), bf)
        sa2[:, S2_A8 : S2_A8 + 512] = to8c(alt)
        sa2[:, S2_ID : S2_ID + P] = ident
        sa2[:, S2_WB8 : S2_WB8 + 256] = to8c(wbt * W_SCALE)
        sa2[:, S2_BE8 : S2_BE8 + 512] = to8c(be * W_SCALE)

        sb = np.zeros((P, SB_W), bf)
        # fold gamma*b_base into the residual (exact, host-side fp32)
        haf = np.ascontiguousarray(
            (h_A[sl] + gamma * b_base[None, :])
            .reshape(2, P, D_A).transpose(1, 0, 2).reshape(P, 512)
        )
        sb[:, SB_HAF : SB_HAF + 1024] = haf.view(bf)
        sb[:, SB_LSC : SB_LSC + 256] = ln_scale.astype(bf)[None, :]
        sb[:, SB_LBI : SB_LBI + 256] = ln_bias.astype(bf)[None, :]

        in_maps.append({"sma": sa, "sma2": sa2, "p8": p8_carrier, "smb": sb})
    return in_maps


def run_kernel(trace=False, **inputs):
    from concourse.bass_utils import run_bass_kernel_spmd

    nc = _get_nc()
    in_maps = make_in_maps(**inputs)
    res = run_bass_kernel_spmd(nc, in_maps, core_ids=list(range(NC_COUNT)), trace=trace)
    out = np.concatenate([r["out"] for r in res.results], axis=0)
    return out.astype(np.float32), res


def kernel(**inputs) -> np.ndarray:
    out, _ = run_kernel(trace=False, **inputs)
    return out


# revision 16
# speedup vs baseline: 1.0054x; 1.0054x over previous
"""Bass/Trainium2 kernel for nn_DWAMiddleLayer (low-rank MoE weight-assembly layer).

Math (reference):
    U    = pool[:, :1024].reshape(N, DB, R)      # [512, 256, 4]
    V    = pool[:, 1024:2048].reshape(N, R, DA)  # [512, 4, 256]
    bE   = pool[:, 2048:2304]                    # [512, 256]
    h_t  = h_A @ W_base.T
           + sum_r (alpha * (h_A @ V_r.T)) @ U_r          # never materialize W_assembled
           + alpha @ bE + b_base
    y    = h_A + gamma * h_t ; out = LayerNorm(y) * ln_scale + ln_bias

Distribution: data-parallel over batch B=2048 across 8 cores (BS=256 rows each);
pool/W_base/vectors replicated.

v5: the whole h_t matmul path runs in fp8-e4m3 DoubleRow (2 k-tiles per
instruction, 2x PE rate, half the pool HBM bytes) into ONE accumulator that
carries a 128x power-of-2 scale (V*64, U*32, alpha*2^-4; bE*128, W_base*128);
the epilogue divides it out during the PSUM->SBUF copy. gamma*b_base is folded
into the fp32 residual h_A on the host (exact), removing the rank-1 matmul.
gamma-scaling of h_t keeps the fp8 error ~1e-3 in the output. Bulk data flows
via SWDGE (gpsimd, ~340GB/s); sa via the sync HWDGE ring in parallel. The PE
is warmed with dummy matmuls until real data arrives so the HAM clock gate
(1.2 vs 2.4 GHz) stays lifted. LN epilogue is pipelined per batch-chunk across
Scalar (normalize via per-partition scale/bias) and Vector, with per-chunk
output DMAs on both HWDGE rings.
"""

import numpy as np

B, N, D_A, D_B, R = 2048, 512, 256, 256, 4
NC_COUNT = 8
BS = B // NC_COUNT  # 256 batch rows per core
P = 128
LN_EPS = 1e-5

V_SCALE = 64.0
U_SCALE = 32.0
A_SCALE = 1.0 / 16.0       # alpha^T pre-scale for the s-path
W_SCALE = 128.0            # W_base^T and bE fp8 scales (match the accumulator)
ACC_SCALE = V_SCALE * U_SCALE * A_SCALE  # = 128: acc8 carries 128 * h_t

# ---- sa1 (bf16 cols; fp8 regions bitcast), needed first (HWDGE) ----
SA_HA8 = 0      # hA^T fp8         [p_a, 2 ach, 256 b]  (256 carrier cols)
SA_ALT = 256    # alpha^T * 2^-4 bf16 [p_n, 4 och, 256 b]
SA_GE = 1280    # fp32 [gamma, eps] bitcast -> 4 bf16 cols
SA_W = 1284
# ---- sa2 (bf16 cols; fp8 regions bitcast), needed mid-stream (SWDGE) ----
S2_A8 = 0       # alpha^T fp8      [p_n, 4 och, 256 b]  (512 carrier cols)
S2_ID = 512     # ident            [p, 128] bf16
S2_WB8 = 640    # W_base^T * 128 fp8 [p_a, 2 ach, 256 c] (256 carrier cols)
S2_BE8 = 896    # bE * 128 fp8     [p_n, 4 o, 256 c]    (512 carrier cols)
S2_W = 1408
# ---- packed small tensor B (bf16 cols), needed late (epilogue) ----
SB_HAF = 0      # (h_A + gamma*b_base) fp32 [p_b, 2 bch, 256 a] -> 1024 bf16 cols
SB_LSC = 1024   # ln_scale  [p, 256] replicated
SB_LBI = 1280   # ln_bias   [p, 256] replicated
SB_W = 1536
# ---- fp8 pool pair layout: d_p8 bf16 [128, 2 pair, 2048] -> fp8 [., ., 4096]
#      per o within pair: [VT 1024 | U2 1024] fp8 cols
P8_VT = 0       # [ach(2), r(4), pn(128)]
P8_U2 = 1024    # [r(4), cch(2), pc(128)]

N_WARM = 9  # warm-up matmuls (j=512): bridge PE activity until data arrives

_cache = {}


def _build_nc():
    import concourse.mybir as mybir
    import concourse.tile as tile
    from concourse import bacc

    fp32 = mybir.dt.float32
    bf16 = mybir.dt.bfloat16
    fp8 = mybir.dt.float8e4
    DR = mybir.MatmulPerfMode.DoubleRow

    nc = bacc.Bacc("TRN2", target_bir_lowering=False)

    # ---- DRAM I/O (per-core shard shapes) ----
    d_sa = nc.dram_tensor("sma", [P, SA_W], bf16, kind="ExternalInput")
    d_sa2 = nc.dram_tensor("sma2", [P, S2_W], bf16, kind="ExternalInput")
    d_p8 = nc.dram_tensor("p8", [P, 2, 2048], bf16, kind="ExternalInput")
    d_sb = nc.dram_tensor("smb", [P, SB_W], bf16, kind="ExternalInput")
    d_out = nc.dram_tensor("out", [BS, D_A], fp32, kind="ExternalOutput")

    with tile.TileContext(nc) as tc:
        with (
            tc.tile_pool(name="persist", bufs=1) as persist,
            tc.tile_pool(name="stage", bufs=2) as stage,
            tc.tile_pool(name="sm", bufs=3) as sm,
            tc.tile_pool(name="pp_t", bufs=2, space="PSUM") as pp_t,
            tc.tile_pool(name="pp_a8", bufs=1, space="PSUM") as pp_a8,
            tc.tile_pool(name="pp_tr", bufs=1, space="PSUM") as pp_tr,
            tc.tile_pool(name="pp_w", bufs=1, space="PSUM") as pp_w,
        ):
            # ---------- PE warm-up: junk matmuls to lift the HAM clock gate ----------
            wsrc = persist.tile([P, 512], bf16)
            nc.vector.memset(wsrc, 0.0)
            warm_ps = pp_w.tile([P, 512], fp32, tag="warm")
            for _ in range(N_WARM):
                nc.tensor.matmul(
                    warm_ps, lhsT=wsrc[:, 0:P], rhs=wsrc, start=True, stop=True,
                    skip_group_check=True,
                )

            # ---------- loads ----------
            # bulk fp8 pool pairs + late smalls via SWDGE (gpsimd, FIFO order);
            # sa via the sync HWDGE ring concurrently.
            p8t = [
                stage.tile([P, 2048], bf16, tag="p8", name=f"p8_{pr}")
                for pr in range(2)
            ]
            # SWDGE (FIFO): pair1 then sb
            nc.gpsimd.dma_start(p8t[1], d_p8[:, 1])
            sb = persist.tile([P, SB_W], bf16)
            nc.gpsimd.dma_start(sb, d_sb[:])
            # HWDGE (fast first-byte): sa1, pair0, sa2 -- the critical-path data
            sa = persist.tile([P, SA_W], bf16)
            nc.sync.dma_start(sa, d_sa[:])
            nc.sync.dma_start(p8t[0], d_p8[:, 0])
            sa2 = persist.tile([P, S2_W], bf16)
            nc.sync.dma_start(sa2, d_sa2[:])

            hA8 = sa[:, SA_HA8 : SA_HA8 + 256].bitcast(fp8).rearrange(
                "p (a b) -> p a b", a=2
            )
            alphaT = sa[:, SA_ALT : SA_ALT + 1024].rearrange("p (o b) -> p o b", o=4)
            a8 = sa2[:, S2_A8 : S2_A8 + 512].bitcast(fp8).rearrange(
                "p (o b) -> p o b", o=4
            )
            ident_b = sa2[:, S2_ID : S2_ID + P]
            Wb8 = sa2[:, S2_WB8 : S2_WB8 + 256].bitcast(fp8).rearrange(
                "p (a c) -> p a c", a=2
            )
            bE8 = sa2[:, S2_BE8 : S2_BE8 + 512].bitcast(fp8).rearrange(
                "p (o c) -> p o c", o=4
            )
            ge = sa[:, SA_GE : SA_GE + 4].bitcast(fp32)
            gamma_col = ge[:, 0:1]
            eps_col = ge[:, 1:2]
            hA_f32 = sb[:, SB_HAF : SB_HAF + 1024].bitcast(fp32).rearrange(
                "p (o a) -> p o a", o=2
            )
            lsc_row = sb[:, SB_LSC : SB_LSC + 256]
            lbi_row = sb[:, SB_LBI : SB_LBI + 256]

            # probe: gpsimd elementwise throughput (junk data, no deps)
            gprobe = sm.tile([P, 512], bf16, tag="gprobe")
            nc.gpsimd.tensor_mul(gprobe, wsrc, wsrc)

            # warm the ACT tables (Copy for the copies, Sqrt for the LN tail)
            warm_act = sm.tile([P, 1], fp32, tag="warmact")
            nc.scalar.activation(
                warm_act, wsrc[:, 0:1], mybir.ActivationFunctionType.Copy
            )
            nc.scalar.activation(
                warm_act, wsrc[:, 0:1], mybir.ActivationFunctionType.Sqrt
            )

            # ---------- h_t^T accumulator (fp8 DoubleRow path, x128 scale) ----------
            acc8 = pp_a8.tile([P, 2, BS], fp32, tag="a8")
            st8 = [False, False]

            def mm8(ch, lhsT, rhs, last=False):
                nc.tensor.matmul(
                    acc8[:, ch], lhsT=lhsT, rhs=rhs,
                    start=(not st8[ch]), stop=last,
                    perf_mode=DR, skip_group_check=True,
                )
                st8[ch] = True

            # ---------- main pipeline over expert-chunk pairs ----------
            for pr in range(2):
                pc8 = p8t[pr].bitcast(fp8).rearrange("p (o f) -> p o f", o=2)
                s8 = sm.tile([P, 2, 4, BS], fp8, tag="s8")
                for oi in range(2):
                    o = pr * 2 + oi
                    VT_o = pc8[:, oi, P8_VT : P8_VT + 1024].rearrange(
                        "p (a r q) -> p a r q", a=2, r=4
                    )
                    # mm1 (DoubleRow, contraction a=256 in one matmul per r)
                    t_ps = pp_t.tile([P, 4, BS], fp32, tag="t")
                    for r in range(4):
                        nc.tensor.matmul(
                            t_ps[:, r],
                            lhsT=VT_o[:, :, r],
                            rhs=hA8,
                            start=True,
                            stop=True,
                            perf_mode=DR,
                        )
                    # s = (alpha * 2^-4) * t : direct-from-PSUM DVE multiply
                    nc.vector.tensor_mul(
                        s8[:, oi], t_ps,
                        alphaT[:, o : o + 1, :].to_broadcast((P, 4, BS)),
                    )
                # bias-mm (DoubleRow): 128*bE^T @ alpha^T, contraction n-pair
                for ch in range(2):
                    mm8(ch, bE8[:, 2 * pr : 2 * pr + 2, ch * P : (ch + 1) * P],
                        a8[:, 2 * pr : 2 * pr + 2])
                if pr == 0:
                    # base-mm (DoubleRow): 128*W_base^T @ hA^T, contraction a
                    for ch in range(2):
                        mm8(ch, Wb8[:, :, ch * P : (ch + 1) * P], hA8)
                # keepalive junk matmuls: PE would idle waiting for s while the
                # DVE multiplies; idle >3.4us re-throttles the HAM clock gate
                for _ in range(8):
                    nc.tensor.matmul(
                        warm_ps[:, 0:P], lhsT=wsrc[:, 0:P], rhs=wsrc[:, 0:P],
                        start=True, stop=True, skip_group_check=True,
                    )
                # mm2 (DoubleRow over the o-pair, contraction n=256)
                U2_pr = pc8[:, :, P8_U2 : P8_U2 + 1024].rearrange(
                    "p o (r c q) -> p o r c q", r=4, c=2
                )
                for r in range(4):
                    for ch in range(2):
                        mm8(ch, U2_pr[:, :, r, ch], s8[:, :, r],
                            last=(pr == 1 and r == 3 and ch == 1))

            # ---------- epilogue: h_t back to batch-major, residual + LN ----------
            ht8 = sm.tile([P, 2, BS], bf16, tag="ht8")
            nc.scalar.activation(
                ht8, acc8, mybir.ActivationFunctionType.Copy, scale=1.0 / ACC_SCALE
            )
            ht_ps = pp_tr.tile([P, 2, D_A], fp32, tag="tr")
            for bch in range(2):
                for cch in range(2):
                    nc.tensor.matmul(
                        ht_ps[:, bch, cch * P : (cch + 1) * P],
                        lhsT=ht8[:, cch, bch * P : (bch + 1) * P],
                        rhs=ident_b,
                        start=True,
                        stop=True,
                        skip_group_check=True,
                    )

            # y = (h_A + gamma*b_base) + gamma * h_t', per-batch-chunk pipeline
            y_sb = sm.tile([P, 2, D_A], fp32, tag="y")
            stats = sm.tile([P, 2, 6], fp32, tag="st")
            mv = sm.tile([P, 2, 2], fp32, tag="mv")
            for bch in range(2):
                nc.vector.scalar_tensor_tensor(
                    y_sb[:, bch],
                    in0=ht_ps[:, bch],
                    scalar=gamma_col,
                    in1=hA_f32[:, bch],
                    op0=mybir.AluOpType.mult,
                    op1=mybir.AluOpType.add,
                )
                nc.vector.bn_stats(stats[:, bch], y_sb[:, bch])
                nc.vector.bn_aggr(mv[:, bch], stats[:, bch])
            # per-batch-chunk: rstd/nmr, normalize on ACT, scale/bias on DVE
            rstd = sm.tile([P, 2], fp32, tag="rstd")
            nmr = sm.tile([P, 2], fp32, tag="nmr")
            w_sb = sm.tile([P, 2, D_A], fp32, tag="w")
            out_sb = sm.tile([P, 2, D_A], fp32, tag="out")
            for bch in range(2):
                nc.scalar.activation(
                    rstd[:, bch : bch + 1],
                    mv[:, bch, 1:2],
                    mybir.ActivationFunctionType.Sqrt,
                    bias=eps_col,
                )
                nc.vector.reciprocal(rstd[:, bch : bch + 1], rstd[:, bch : bch + 1])
                nc.vector.scalar_tensor_tensor(
                    nmr[:, bch : bch + 1],
                    in0=mv[:, bch, 0:1],
                    scalar=-1.0,
                    in1=rstd[:, bch : bch + 1],
                    op0=mybir.AluOpType.mult,
                    op1=mybir.AluOpType.mult,
                )
                nc.scalar.activation(
                    w_sb[:, bch],
                    y_sb[:, bch],
                    mybir.ActivationFunctionType.Identity,
                    bias=nmr[:, bch : bch + 1],
                    scale=rstd[:, bch : bch + 1],
                )
                nc.vector.tensor_mul(w_sb[:, bch], w_sb[:, bch], lsc_row)
                nc.vector.tensor_add(out_sb[:, bch], w_sb[:, bch], lbi_row)
                eng = nc.sync if bch == 0 else nc.scalar
                eng.dma_start(d_out[bch * P : (bch + 1) * P, :], out_sb[:, bch])

    nc.compile()
    return nc


def _get_nc():
    if "nc" not in _cache:
        _cache["nc"] = _build_nc()
    return _cache["nc"]


def make_in_maps(**inputs):
    """Shard + pre-transpose + pre-cast full inputs into 8 per-core input maps."""
    import ml_dtypes

    bf = ml_dtypes.bfloat16
    f8 = ml_dtypes.float8_e4m3fn
    f32 = lambda x: np.ascontiguousarray(np.asarray(x), dtype=np.float32)

    def to8c(x):  # fp8 bytes packed into a bf16 bit-carrier, 2 per column
        q = np.clip(x, -240.0, 240.0).astype(f8)  # TRN e4m3 tops out at +-240
        return q.reshape(q.shape[0], -1).view(np.uint8).view(np.uint16).view(bf)

    h_A = f32(inputs["h_A"])
    alpha = f32(inputs["alpha"])
    pool = np.asarray(inputs["pool_vectors"], dtype=np.float32)
    W_base = f32(inputs["W_base"])
    b_base = f32(inputs["b_base"]).reshape(D_B)
    gamma = float(np.asarray(inputs["gamma"]).reshape(()))
    ln_scale = f32(inputs["ln_scale"]).reshape(D_A)
    ln_bias = f32(inputs["ln_bias"]).reshape(D_A)

    U = pool[:, : D_B * R].reshape(N, D_B, R)
    V = pool[:, D_B * R : D_B * R + R * D_A].reshape(N, R, D_A)
    bE = pool[:, D_B * R + R * D_A : D_B * R + R * D_A + D_B]

    # fp8 pool pairs: [p, pair, o_in_pair, [VT | U2]] packed as bf16 bit-carrier
    p8 = np.empty((P, 2, 2, 2048), np.float32)
    for o in range(4):
        nsl = slice(o * P, (o + 1) * P)
        vt = V[nsl].transpose(2, 1, 0).reshape(2, P, R, P).transpose(1, 0, 2, 3)
        p8[:, o // 2, o % 2, P8_VT : P8_VT + 1024] = vt.reshape(P, 1024) * V_SCALE
        u2 = U[nsl].transpose(0, 2, 1).reshape(P, R, 2, P)
        p8[:, o // 2, o % 2, P8_U2 : P8_U2 + 1024] = u2.reshape(P, 1024) * U_SCALE
    p8_carrier = to8c(p8.reshape(P, -1)).reshape(P, 2, 2048)

    ident = np.eye(P, dtype=np.float32).astype(bf)
    ge = np.empty((P, 2), np.float32)
    ge[:, 0] = gamma
    ge[:, 1] = LN_EPS
    wbt = np.ascontiguousarray(
        W_base.T.reshape(2, P, D_B).transpose(1, 0, 2).reshape(P, 512)
    )
    be = np.ascontiguousarray(
        bE.reshape(4, P, D_B).transpose(1, 0, 2).reshape(P, 1024)
    )

    in_maps = []
    for i in range(NC_COUNT):
        sl = slice(i * BS, (i + 1) * BS)
        hat = h_A[sl].T.reshape(2, P, BS).transpose(1, 0, 2).reshape(P, 512)
        alt = alpha[sl].T.reshape(4, P, BS).transpose(1, 0, 2).reshape(P, 1024)

        sa = np.zeros((P, SA_W), bf)
        sa[:, SA_HA8 : SA_HA8 + 256] = to8c(hat)
        sa[:, SA_ALT : SA_ALT + 1024] = (alt * A_SCALE).astype(bf)
        sa[:, SA_GE : SA_GE + 4] = ge.view(bf)
        sa2 = np.zeros((P, S2_W), bf)
        sa2[:, S2_A8 : S2_A8 + 512] = to8c(alt)
        sa2[:, S2_ID : S2_ID + P] = ident
        sa2[:, S2_WB8 : S2_WB8 + 256] = to8c(wbt * W_SCALE)
        sa2[:, S2_BE8 : S2_BE8 + 512] = to8c(be * W_SCALE)

        sb = np.zeros((P, SB_W), bf)
        # fold gamma*b_base into the residual (exact, host-side fp32)
        haf = np.ascontiguousarray(
            (h_A[sl] + gamma * b_base[None, :])
            .reshape(2, P, D_A).transpose(1, 0, 2).reshape(P, 512)
        )
        sb[:, SB_HAF : SB_HAF + 1024] = haf.view(bf)
        sb[:, SB_LSC : SB_LSC + 256] = ln_scale.astype(bf)[None, :]
        sb[:, SB_LBI : SB_LBI + 256] = ln_bias.astype(bf)[None, :]

        in_maps.append({"sma": sa, "sma2": sa2, "p8": p8_carrier, "smb": sb})
    return in_maps


def run_kernel(trace=False, **inputs):
    from concourse.bass_utils import run_bass_kernel_spmd

    nc = _get_nc()
    in_maps = make_in_maps(**inputs)
    res = run_bass_kernel_spmd(nc, in_maps, core_ids=list(range(NC_COUNT)), trace=trace)
    out = np.concatenate([r["out"] for r in res.results], axis=0)
    return out.astype(np.float32), res


def kernel(**inputs) -> np.ndarray:
    out, _ = run_kernel(trace=False, **inputs)
    return out


# revision 18
# speedup vs baseline: 1.0399x; 1.0344x over previous
"""Bass/Trainium2 kernel for nn_DWAMiddleLayer (low-rank MoE weight-assembly layer).

Math (reference):
    U    = pool[:, :1024].reshape(N, DB, R)      # [512, 256, 4]
    V    = pool[:, 1024:2048].reshape(N, R, DA)  # [512, 4, 256]
    bE   = pool[:, 2048:2304]                    # [512, 256]
    h_t  = h_A @ W_base.T
           + sum_r (alpha * (h_A @ V_r.T)) @ U_r          # never materialize W_assembled
           + alpha @ bE + b_base
    y    = h_A + gamma * h_t ; out = LayerNorm(y) * ln_scale + ln_bias

Distribution: data-parallel over batch B=2048 across 8 cores (BS=256 rows each);
pool/W_base/vectors replicated.

v5: the whole h_t matmul path runs in fp8-e4m3 DoubleRow (2 k-tiles per
instruction, 2x PE rate, half the pool HBM bytes) into ONE accumulator that
carries a 128x power-of-2 scale (V*64, U*32, alpha*2^-4; bE*128, W_base*128);
the epilogue divides it out during the PSUM->SBUF copy. gamma*b_base is folded
into the fp32 residual h_A on the host (exact), removing the rank-1 matmul.
gamma-scaling of h_t keeps the fp8 error ~1e-3 in the output. Bulk data flows
via SWDGE (gpsimd, ~340GB/s); sa via the sync HWDGE ring in parallel. The PE
is warmed with dummy matmuls until real data arrives so the HAM clock gate
(1.2 vs 2.4 GHz) stays lifted. LN epilogue is pipelined per batch-chunk across
Scalar (normalize via per-partition scale/bias) and Vector, with per-chunk
output DMAs on both HWDGE rings.
"""

import numpy as np

B, N, D_A, D_B, R = 2048, 512, 256, 256, 4
NC_COUNT = 8
BS = B // NC_COUNT  # 256 batch rows per core
P = 128
LN_EPS = 1e-5

V_SCALE = 64.0
U_SCALE = 32.0
A_SCALE = 1.0 / 16.0       # alpha^T pre-scale for the s-path
W_SCALE = 128.0            # W_base^T and bE fp8 scales (match the accumulator)
ACC_SCALE = V_SCALE * U_SCALE * A_SCALE  # = 128: acc8 carries 128 * h_t

# ---- sa1 (bf16 cols; fp8 regions bitcast), needed first (HWDGE) ----
SA_HA8 = 0      # hA^T fp8         [p_a, 2 ach, 256 b]  (256 carrier cols)
SA_ALT = 256    # alpha^T * 2^-4 bf16 [p_n, 4 och, 256 b]
SA_GE = 1280    # fp32 [gamma, eps] bitcast -> 4 bf16 cols
SA_W = 1284
# ---- sa2 (bf16 cols; fp8 regions bitcast), needed mid-stream (SWDGE) ----
S2_A8 = 0       # alpha^T fp8      [p_n, 4 och, 256 b]  (512 carrier cols)
S2_ID = 512     # ident            [p, 128] bf16
S2_WB8 = 640    # W_base^T * 128 fp8 [p_a, 2 ach, 256 c] (256 carrier cols)
S2_BE8 = 896    # bE * 128 fp8     [p_n, 4 o, 256 c]    (512 carrier cols)
S2_W = 1408
# ---- packed small tensor B (bf16 cols), needed late (epilogue) ----
SB_HAF = 0      # (h_A + gamma*b_base) fp32 [p_b, 2 bch, 256 a] -> 1024 bf16 cols
SB_LSC = 1024   # ln_scale  [p, 256] replicated
SB_LBI = 1280   # ln_bias   [p, 256] replicated
SB_W = 1536
# ---- fp8 pool pair layout: d_p8 bf16 [128, 2 pair, 2048] -> fp8 [., ., 4096]
#      per o within pair: [VT 1024 | U2 1024] fp8 cols
P8_VT = 0       # [ach(2), r(4), pn(128)]
P8_U2 = 1024    # [r(4), cch(2), pc(128)]

N_WARM = 9  # warm-up matmuls (j=512): bridge PE activity until data arrives

_cache = {}


def _build_nc():
    import concourse.mybir as mybir
    import concourse.tile as tile
    from concourse import bacc

    fp32 = mybir.dt.float32
    bf16 = mybir.dt.bfloat16
    fp8 = mybir.dt.float8e4
    DR = mybir.MatmulPerfMode.DoubleRow

    nc = bacc.Bacc("TRN2", target_bir_lowering=False)

    # ---- DRAM I/O (per-core shard shapes) ----
    d_sa = nc.dram_tensor("sma", [P, SA_W], bf16, kind="ExternalInput")
    d_sa2 = nc.dram_tensor("sma2", [P, S2_W], bf16, kind="ExternalInput")
    d_p8 = nc.dram_tensor("p8", [P, 2, 2048], bf16, kind="ExternalInput")
    d_sb = nc.dram_tensor("smb", [P, SB_W], bf16, kind="ExternalInput")
    d_out = nc.dram_tensor("out", [BS, D_A], fp32, kind="ExternalOutput")

    with tile.TileContext(nc) as tc:
        with (
            tc.tile_pool(name="persist", bufs=1) as persist,
            tc.tile_pool(name="stage", bufs=2) as stage,
            tc.tile_pool(name="sm", bufs=3) as sm,
            tc.tile_pool(name="pp_t", bufs=2, space="PSUM") as pp_t,
            tc.tile_pool(name="pp_a8", bufs=1, space="PSUM") as pp_a8,
            tc.tile_pool(name="pp_tr", bufs=1, space="PSUM") as pp_tr,
            tc.tile_pool(name="pp_w", bufs=1, space="PSUM") as pp_w,
        ):
            # ---------- PE warm-up: junk matmuls to lift the HAM clock gate ----------
            wsrc = persist.tile([P, 512], bf16)
            nc.vector.memset(wsrc, 0.0)
            warm_ps = pp_w.tile([P, 512], fp32, tag="warm")
            for _ in range(N_WARM):
                nc.tensor.matmul(
                    warm_ps, lhsT=wsrc[:, 0:P], rhs=wsrc, start=True, stop=True,
                    skip_group_check=True,
                )

            # ---------- loads ----------
            # bulk fp8 pool pairs + late smalls via SWDGE (gpsimd, FIFO order);
            # sa via the sync HWDGE ring concurrently.
            p8t = [
                stage.tile([P, 2048], bf16, tag="p8", name=f"p8_{pr}")
                for pr in range(2)
            ]
            # SWDGE (FIFO): sa2 then sb (mid/late-needed)
            sa2 = persist.tile([P, S2_W], bf16)
            nc.gpsimd.dma_start(sa2, d_sa2[:])
            sb = persist.tile([P, SB_W], bf16)
            nc.gpsimd.dma_start(sb, d_sb[:])
            # HWDGE, uncontended: sa1, pair0, pair1 -- the critical-path data
            sa = persist.tile([P, SA_W], bf16)
            nc.sync.dma_start(sa, d_sa[:])
            nc.sync.dma_start(p8t[0], d_p8[:, 0])
            nc.sync.dma_start(p8t[1], d_p8[:, 1])

            hA8 = sa[:, SA_HA8 : SA_HA8 + 256].bitcast(fp8).rearrange(
                "p (a b) -> p a b", a=2
            )
            alphaT = sa[:, SA_ALT : SA_ALT + 1024].rearrange("p (o b) -> p o b", o=4)
            a8 = sa2[:, S2_A8 : S2_A8 + 512].bitcast(fp8).rearrange(
                "p (o b) -> p o b", o=4
            )
            ident_b = sa2[:, S2_ID : S2_ID + P]
            Wb8 = sa2[:, S2_WB8 : S2_WB8 + 256].bitcast(fp8).rearrange(
                "p (a c) -> p a c", a=2
            )
            bE8 = sa2[:, S2_BE8 : S2_BE8 + 512].bitcast(fp8).rearrange(
                "p (o c) -> p o c", o=4
            )
            ge = sa[:, SA_GE : SA_GE + 4].bitcast(fp32)
            gamma_col = ge[:, 0:1]
            eps_col = ge[:, 1:2]
            hA_f32 = sb[:, SB_HAF : SB_HAF + 1024].bitcast(fp32).rearrange(
                "p (o a) -> p o a", o=2
            )
            lsc_row = sb[:, SB_LSC : SB_LSC + 256]
            lbi_row = sb[:, SB_LBI : SB_LBI + 256]

            # probe: gpsimd elementwise throughput (junk data, no deps)
            gprobe = sm.tile([P, 512], bf16, tag="gprobe")
            nc.gpsimd.tensor_mul(gprobe, wsrc, wsrc)

            # warm the ACT tables (Copy for the copies, Sqrt for the LN tail)
            warm_act = sm.tile([P, 1], fp32, tag="warmact")
            nc.scalar.activation(
                warm_act, wsrc[:, 0:1], mybir.ActivationFunctionType.Copy
            )
            nc.scalar.activation(
                warm_act, wsrc[:, 0:1], mybir.ActivationFunctionType.Sqrt
            )

            # ---------- h_t^T accumulator (fp8 DoubleRow path, x128 scale) ----------
            acc8 = pp_a8.tile([P, 2, BS], fp32, tag="a8")
            st8 = [False, False]

            def mm8(ch, lhsT, rhs, last=False):
                nc.tensor.matmul(
                    acc8[:, ch], lhsT=lhsT, rhs=rhs,
                    start=(not st8[ch]), stop=last,
                    perf_mode=DR, skip_group_check=True,
                )
                st8[ch] = True

            # ---------- main pipeline over expert-chunk pairs ----------
            for pr in range(2):
                pc8 = p8t[pr].bitcast(fp8).rearrange("p (o f) -> p o f", o=2)
                s8 = sm.tile([P, 2, 4, BS], fp8, tag="s8")
                for oi in range(2):
                    o = pr * 2 + oi
                    VT_o = pc8[:, oi, P8_VT : P8_VT + 1024].rearrange(
                        "p (a r q) -> p a r q", a=2, r=4
                    )
                    # mm1 (DoubleRow, contraction a=256 in one matmul per r)
                    t_ps = pp_t.tile([P, 4, BS], fp32, tag="t")
                    for r in range(4):
                        nc.tensor.matmul(
                            t_ps[:, r],
                            lhsT=VT_o[:, :, r],
                            rhs=hA8,
                            start=True,
                            stop=True,
                            perf_mode=DR,
                        )
                    # s = (alpha * 2^-4) * t : direct-from-PSUM DVE multiply
                    nc.vector.tensor_mul(
                        s8[:, oi], t_ps,
                        alphaT[:, o : o + 1, :].to_broadcast((P, 4, BS)),
                    )
                # keepalive junk matmuls: PE would idle waiting for s while the
                # DVE multiplies; idle >3.4us re-throttles the HAM clock gate
                for _ in range(8):
                    nc.tensor.matmul(
                        warm_ps[:, 0:P], lhsT=wsrc[:, 0:P], rhs=wsrc[:, 0:P],
                        start=True, stop=True, skip_group_check=True,
                    )
                # mm2 (DoubleRow over the o-pair, contraction n=256)
                U2_pr = pc8[:, :, P8_U2 : P8_U2 + 1024].rearrange(
                    "p o (r c q) -> p o r c q", r=4, c=2
                )
                for r in range(4):
                    for ch in range(2):
                        mm8(ch, U2_pr[:, :, r, ch], s8[:, :, r])

            # bias-mm (DoubleRow): 128*bE^T @ alpha^T, contraction n-pairs;
            # base-mm: 128*W_base^T @ hA^T -- last in PE order (sa2 arrives mid)
            for pr in range(2):
                for ch in range(2):
                    mm8(ch, bE8[:, 2 * pr : 2 * pr + 2, ch * P : (ch + 1) * P],
                        a8[:, 2 * pr : 2 * pr + 2])
            for ch in range(2):
                mm8(ch, Wb8[:, :, ch * P : (ch + 1) * P], hA8,
                    last=True)

            # ---------- epilogue: h_t back to batch-major, residual + LN ----------
            ht8 = sm.tile([P, 2, BS], bf16, tag="ht8")
            nc.scalar.activation(
                ht8, acc8, mybir.ActivationFunctionType.Copy, scale=1.0 / ACC_SCALE
            )
            ht_ps = pp_tr.tile([P, 2, D_A], fp32, tag="tr")
            for bch in range(2):
                for cch in range(2):
                    nc.tensor.matmul(
                        ht_ps[:, bch, cch * P : (cch + 1) * P],
                        lhsT=ht8[:, cch, bch * P : (bch + 1) * P],
                        rhs=ident_b,
                        start=True,
                        stop=True,
                        skip_group_check=True,
                    )

            # y = (h_A + gamma*b_base) + gamma * h_t', per-batch-chunk pipeline
            y_sb = sm.tile([P, 2, D_A], fp32, tag="y")
            stats = sm.tile([P, 2, 6], fp32, tag="st")
            mv = sm.tile([P, 2, 2], fp32, tag="mv")
            for bch in range(2):
                nc.vector.scalar_tensor_tensor(
                    y_sb[:, bch],
                    in0=ht_ps[:, bch],
                    scalar=gamma_col,
                    in1=hA_f32[:, bch],
                    op0=mybir.AluOpType.mult,
                    op1=mybir.AluOpType.add,
                )
                nc.vector.bn_stats(stats[:, bch], y_sb[:, bch])
                nc.vector.bn_aggr(mv[:, bch], stats[:, bch])
            # per-batch-chunk: rstd/nmr, normalize on ACT, scale/bias on DVE
            rstd = sm.tile([P, 2], fp32, tag="rstd")
            nmr = sm.tile([P, 2], fp32, tag="nmr")
            w_sb = sm.tile([P, 2, D_A], fp32, tag="w")
            out_sb = sm.tile([P, 2, D_A], fp32, tag="out")
            for bch in range(2):
                nc.scalar.activation(
                    rstd[:, bch : bch + 1],
                    mv[:, bch, 1:2],
                    mybir.ActivationFunctionType.Sqrt,
                    bias=eps_col,
                )
                nc.vector.reciprocal(rstd[:, bch : bch + 1], rstd[:, bch : bch + 1])
                nc.vector.scalar_tensor_tensor(
                    nmr[:, bch : bch + 1],
                    in0=mv[:, bch, 0:1],
                    scalar=-1.0,
                    in1=rstd[:, bch : bch + 1],
                    op0=mybir.AluOpType.mult,
                    op1=mybir.AluOpType.mult,
                )
                nc.scalar.activation(
                    w_sb[:, bch],
                    y_sb[:, bch],
                    mybir.ActivationFunctionType.Identity,
                    bias=nmr[:, bch : bch + 1],
                    scale=rstd[:, bch : bch + 1],
                )
                nc.vector.tensor_mul(w_sb[:, bch], w_sb[:, bch], lsc_row)
                nc.vector.tensor_add(out_sb[:, bch], w_sb[:, bch], lbi_row)
                eng = nc.sync if bch == 0 else nc.scalar
                eng.dma_start(d_out[bch * P : (bch + 1) * P, :], out_sb[:, bch])

    nc.compile()
    return nc


def _get_nc():
    if "nc" not in _cache:
        _cache["nc"] = _build_nc()
    return _cache["nc"]


def make_in_maps(**inputs):
    """Shard + pre-transpose + pre-cast full inputs into 8 per-core input maps."""
    import ml_dtypes

    bf = ml_dtypes.bfloat16
    f8 = ml_dtypes.float8_e4m3fn
    f32 = lambda x: np.ascontiguousarray(np.asarray(x), dtype=np.float32)

    def to8c(x):  # fp8 bytes packed into a bf16 bit-carrier, 2 per column
        q = np.clip(x, -240.0, 240.0).astype(f8)  # TRN e4m3 tops out at +-240
        return q.reshape(q.shape[0], -1).view(np.uint8).view(np.uint16).view(bf)

    h_A = f32(inputs["h_A"])
    alpha = f32(inputs["alpha"])
    pool = np.asarray(inputs["pool_vectors"], dtype=np.float32)
    W_base = f32(inputs["W_base"])
    b_base = f32(inputs["b_base"]).reshape(D_B)
    gamma = float(np.asarray(inputs["gamma"]).reshape(()))
    ln_scale = f32(inputs["ln_scale"]).reshape(D_A)
    ln_bias = f32(inputs["ln_bias"]).reshape(D_A)

    U = pool[:, : D_B * R].reshape(N, D_B, R)
    V = pool[:, D_B * R : D_B * R + R * D_A].reshape(N, R, D_A)
    bE = pool[:, D_B * R + R * D_A : D_B * R + R * D_A + D_B]

    # fp8 pool pairs: [p, pair, o_in_pair, [VT | U2]] packed as bf16 bit-carrier
    p8 = np.empty((P, 2, 2, 2048), np.float32)
    for o in range(4):
        nsl = slice(o * P, (o + 1) * P)
        vt = V[nsl].transpose(2, 1, 0).reshape(2, P, R, P).transpose(1, 0, 2, 3)
        p8[:, o // 2, o % 2, P8_VT : P8_VT + 1024] = vt.reshape(P, 1024) * V_SCALE
        u2 = U[nsl].transpose(0, 2, 1).reshape(P, R, 2, P)
        p8[:, o // 2, o % 2, P8_U2 : P8_U2 + 1024] = u2.reshape(P, 1024) * U_SCALE
    p8_carrier = to8c(p8.reshape(P, -1)).reshape(P, 2, 2048)

    ident = np.eye(P, dtype=np.float32).astype(bf)
    ge = np.empty((P, 2), np.float32)
    ge[:, 0] = gamma
    ge[:, 1] = LN_EPS
    wbt = np.ascontiguousarray(
        W_base.T.reshape(2, P, D_B).transpose(1, 0, 2).reshape(P, 512)
    )
    be = np.ascontiguousarray(
        bE.reshape(4, P, D_B).transpose(1, 0, 2).reshape(P, 1024)
    )

    in_maps = []
    for i in range(NC_COUNT):
        sl = slice(i * BS, (i + 1) * BS)
        hat = h_A[sl].T.reshape(2, P, BS).transpose(1, 0, 2).reshape(P, 512)
        alt = alpha[sl].T.reshape(4, P, BS).transpose(1, 0, 2).reshape(P, 1024)

        sa = np.zeros((P, SA_W), bf)
        sa[:, SA_HA8 : SA_HA8 + 256] = to8c(hat)
        sa[:, SA_ALT : SA_ALT + 1024] = (alt * A_SCALE).astype(bf)
        sa[:, SA_GE : SA_GE + 4] = ge.view(bf)
        sa2 = np.zeros((P, S2_W), bf)
        sa2[:, S2_A8 : S2_A8 + 512] = to8c(alt)
        sa2[:, S2_ID : S2_ID + P] = ident
        sa2[:, S2_WB8 : S2_WB8 + 256] = to8c(wbt * W_SCALE)
        sa2[:, S2_BE8 : S2_BE8 + 512] = to8c(be * W_SCALE)

        sb = np.zeros((P, SB_W), bf)
        # fold gamma*b_base into the residual (exact, host-side fp32)
        haf = np.ascontiguousarray(
            (h_A[sl] + gamma * b_base[None, :])
            .reshape(2, P, D_A).transpose(1, 0, 2).reshape(P, 512)
        )
        sb[:, SB_HAF : SB_HAF + 1024] = haf.view(bf)
        sb[:, SB_LSC : SB_LSC + 256] = ln_scale.astype(bf)[None, :]
        sb[:, SB_LBI : SB_LBI + 256] = ln_bias.astype(bf)[None, :]

        in_maps.append({"sma": sa, "sma2": sa2, "p8": p8_carrier, "smb": sb})
    return in_maps


def run_kernel(trace=False, **inputs):
    from concourse.bass_utils import run_bass_kernel_spmd

    nc = _get_nc()
    in_maps = make_in_maps(**inputs)
    res = run_bass_kernel_spmd(nc, in_maps, core_ids=list(range(NC_COUNT)), trace=trace)
    out = np.concatenate([r["out"] for r in res.results], axis=0)
    return out.astype(np.float32), res


def kernel(**inputs) -> np.ndarray:
    out, _ = run_kernel(trace=False, **inputs)
    return out
